# revision 53
# baseline (speedup 1.0000x reference)
"""AttentiveMatchingLayer TRN2 kernel (skewed software pipeline).

Math (per batch, validated against the jax reference):
  ssa[t] = sum_d a[t,d]^2 ; ssb likewise ; stok = 1/sqrt(ssa*ssb)
  as = a * stok[:,None]                     # carries BOTH l2 norms
  alpha[d,e] = sum_t b[t,d] * as[t,e]       # == ref alpha (norms folded)
  sal[e] = 1/sqrt(sum_d alpha[d,e]^2)
  hm[e,t] = sal[e] * sum_d alpha[d,e] * b[t,d]
     (differs from ref hmean by a per-token positive factor, which
      cancels in the final cosine; sal is folded into hm's PSUM
      evacuation scale so all final matmuls share the plain w2 rhs)
  num[t,p] = sum_e (aT*hm)[e,t] w2[e,p] ; sa = sum_d a^2 w2 (+ ones col
  -> ssa) ; sh = sum_e hm^2 w2
  persp = num / sqrt(sa*sh)
Sharding: data-parallel over batch B=32 across 8 cores (4 batches/core).

Implementation notes (engine assignment calibrated on the CoreSim cost
model; TimelineSim 51.2us/core vs 68.1us for the previous version):
- inputs pre-cast to f16 on host; all loads/stores ride the two HWDGE
  queues (sync + scalar) at half-batch granularity -- no SWDGE
  descriptor-generation cost on any engine (994ns fixed/dma on Pool).
- ssb rides the transposed path: bsq = bT^2 (DVE 2x mode) matmul'd
  against a ones column (PE) -- no [t,d]-layout square+reduce (the
  baseline spent 16.6us Pool + 8.8us DVE there).
- per-batch phases are emitted SKEWED (batch b runs phase s-b at step
  s), shallow-first within a step: each engine's strictly in-order
  stream reaches ready work (younger batches' transposes) before
  chain-gated deeper phases.
- engine budget (per core, measured balanced 30.6/30.6): ACT = asq
  (Square directly from transpose PSUM) + bT-dc0/alpha/hm/sa
  evacuations + sqrts; DVE = aT/bT-dc1 evacuations (TensorCopy 2x
  from f16 PSUM) + bsq/alsq/hmsq/prod muls (2x SBUF) + as (4x) +
  stok/sal/den/persp chains;
  Pool 21.5us = as-odd-chunks + prod/hmsq t2=1 (SBUF-only engine, 2x
  slower than DVE, so the last batch runs all-DVE to shorten the
  tail); PE 24.9us = transposes + alpha/hm/sa/num/sh matmuls.
- PSUM discipline: at most ONE open accumulation chain per bank
  (interleaved start/stop groups in a shared bank corrupt results);
  generic 8-slot pool outperformed per-tag tight allocations.
- stok/sal scalar chains run per batch (cross-batch batching of the
  sqrt/recip serialized the whole pipeline for ~10us).
"""

import numpy as np
from contextlib import ExitStack

import concourse.bacc as bacc
import concourse.bass as bass
import concourse.tile as tile
from concourse import masks, mybir

B, T, D, P = 32, 1024, 256, 20
PA = P + 1         # w2t augmented with a ones column (-> ssa)
N_CORES = 8
NB = B // N_CORES  # batches per core
TC = T // 128      # 8 token chunks
DC = D // 128      # 2 d chunks
F32 = mybir.dt.float32
F16 = mybir.dt.float16
Sqrt = mybir.ActivationFunctionType.Sqrt
Copy = mybir.ActivationFunctionType.Copy
Square = mybir.ActivationFunctionType.Square


def build_kernel():
    nc = bacc.Bacc("TRN2", target_bir_lowering=False, debug=False,
                   num_devices=N_CORES)
    a_in = nc.declare_dram_parameter("a", [NB, T, D], F16, isOutput=False)
    b_in = nc.declare_dram_parameter("b", [NB, T, D], F16, isOutput=False)
    w2t_in = nc.declare_dram_parameter("w2t", [D, PA], F16, isOutput=False)
    out_d = nc.declare_dram_parameter("out", [NB, T, P], F32, isOutput=True)

    NBR = range(NB)
    with tile.TileContext(nc) as tc, ExitStack() as ctx, \
            nc.allow_low_precision(reason="f16 intermediates; 2e-2 rel tol"):
        consts = ctx.enter_context(tc.tile_pool(name="consts", bufs=1))
        p4 = ctx.enter_context(tc.tile_pool(name="p4", bufs=NB))
        pscr = ctx.enter_context(tc.tile_pool(name="pscr", bufs=4))
        ps = ctx.enter_context(tc.tile_pool(name="ps", bufs=8, space="PSUM"))

        identf = consts.tile([128, 128], F32)
        masks.make_identity(nc, identf[:])
        ident = consts.tile([128, 128], F16)
        nc.vector.tensor_copy(ident[:], identf[:])
        ones = consts.tile([128, 1], F16)
        nc.vector.memset(ones[:], 1.0)
        w2t = consts.tile([128, DC, PA], F16)

        stok_raw = consts.tile([128, NB, TC], F32)
        sal_all = consts.tile([128, NB, 2], F32)
        den_all = consts.tile([128, NB, 2, (TC // 2) * P], F16)

        # ---- loads (f16, HWDGE, half-batch granularity) ----
        a_sb = [p4.tile([128, TC, D], F16, tag="a_sb", name=f"a_sb{b}")
                for b in NBR]
        b_sb = [p4.tile([128, TC, D], F16, tag="b_sb", name=f"b_sb{b}")
                for b in NBR]
        H = TC // 2

        def pload(b):
            # half-batch granularity
            nq = 2
            Q = TC // nq
            for hf in range(nq):
                nc.sync.dma_start(
                    out=a_sb[b][:, hf * Q:(hf + 1) * Q, :],
                    in_=a_in.ap()[b].rearrange(
                        "(p c) d -> p c d", p=128)[:, hf * Q:(hf + 1) * Q, :])
                nc.scalar.dma_start(
                    out=b_sb[b][:, hf * Q:(hf + 1) * Q, :],
                    in_=b_in.ap()[b].rearrange(
                        "(p c) d -> p c d", p=128)[:, hf * Q:(hf + 1) * Q, :])
            if b == 0:
                nc.scalar.dma_start(
                    out=w2t[:],
                    in_=w2t_in.ap().rearrange("(dc p) w -> p dc w", p=128))

        aT_sb = [p4.tile([128, DC, T], F16, tag="aT_sb", name=f"aT{b}")
                 for b in NBR]
        bT_sb = [p4.tile([128, DC, T], F16, tag="bT_sb", name=f"bT{b}")
                 for b in NBR]
        asq_sb = [p4.tile([128, DC, T], F16, tag="asq_sb", name=f"asq{b}")
                  for b in NBR]
        bsq_sb = [p4.tile([128, DC, T], F16, tag="bsq_sb", name=f"bsq{b}")
                  for b in NBR]
        sa_sb = [p4.tile([128, 2, (TC // 2) * PA], F32, tag="sa_sb",
                         name=f"sa_sb{b}") for b in NBR]
        as_sb = [p4.tile([128, TC, D], F16, tag="as_sb", name=f"as_sb{b}")
                 for b in NBR]
        alpha_sb = [p4.tile([128, DC, 256], F16, tag="alpha_sb",
                            name=f"alpha_sb{b}") for b in NBR]
        alsq_sb = [p4.tile([128, DC, 256], F16, tag="alsq_sb",
                           name=f"alsq_sb{b}") for b in NBR]
        prod_sb = [p4.tile([128, 2, T], F16, tag="prod_sb", name=f"prod{b}")
                   for b in NBR]
        hmsq_sb = [p4.tile([128, 2, T], F16, tag="hmsq_sb", name=f"hmsq{b}")
                   for b in NBR]
        fin_ps_all = [None] * NB

        def p0(b):
            # a-transposes first; evacs interleaved per dc-group so asq
            # (ACT, direct from PSUM) starts after 8 transposes, not 16
            aT_ps = [ps.tile([128, 1024], F16, tag="ps",
                             name=f"aT_ps{b}_{i}") for i in range(DC)]
            for dc in range(DC):
                for c in range(TC):
                    nc.tensor.transpose(
                        out=aT_ps[dc][:, c * 128:(c + 1) * 128],
                        in_=a_sb[b][:, c, dc * 128:(dc + 1) * 128],
                        identity=ident[:])
                if b == 0 and dc == 1:
                    # ramp-in: DVE is idle; shorten ACT's serial asq pair
                    nc.vector.tensor_copy(aT_sb[b][:, dc, :], aT_ps[dc][:])
                    nc.vector.tensor_mul(asq_sb[b][:, dc, :],
                                         aT_sb[b][:, dc, :],
                                         aT_sb[b][:, dc, :])
                else:
                    nc.scalar.activation(asq_sb[b][:, dc, :], aT_ps[dc][:],
                                         Square)
                    nc.vector.tensor_copy(aT_sb[b][:, dc, :], aT_ps[dc][:])
            bT_ps = [ps.tile([128, 1024], F16, tag="ps",
                             name=f"bT_ps{b}_{i}") for i in range(DC)]
            for dc in range(DC):
                for c in range(TC):
                    nc.tensor.transpose(
                        out=bT_ps[dc][:, c * 128:(c + 1) * 128],
                        in_=b_sb[b][:, c, dc * 128:(dc + 1) * 128],
                        identity=ident[:])
                if dc == 0:
                    nc.scalar.copy(bT_sb[b][:, dc, :], bT_ps[dc][:])
                else:
                    nc.vector.tensor_copy(bT_sb[b][:, dc, :], bT_ps[dc][:])
                nc.vector.tensor_mul(bsq_sb[b][:, dc, :], bT_sb[b][:, dc, :],
                                     bT_sb[b][:, dc, :])

        def p1(b):
            sa_ps = [ps.tile([128, (TC // 2) * PA], F32, tag="ps",
                             name=f"sa_ps{b}_{i}") for i in range(2)]
            sb_ps = ps.tile([128, TC], F32, tag="ps", name=f"ssb_ps{b}")
            for c in range(TC):
                for dc in range(DC):
                    nc.tensor.matmul(
                        sa_ps[c % 2][:, (c // 2) * PA:(c // 2) * PA + PA],
                        lhsT=asq_sb[b][:, dc, c * 128:(c + 1) * 128],
                        rhs=w2t[:, dc, :],
                        start=(dc == 0), stop=(dc == DC - 1))
                    nc.tensor.matmul(
                        sb_ps[:, c:c + 1],
                        lhsT=bsq_sb[b][:, dc, c * 128:(c + 1) * 128],
                        rhs=ones[:],
                        start=(dc == 0), stop=(dc == DC - 1))
            for h in range(2):
                nc.scalar.copy(sa_sb[b][:, h, :], sa_ps[h][:])
            # stok = 1/sqrt(ssa*ssb); ssa = PA-th col of each group; c=2j+h
            nc.vector.tensor_mul(
                stok_raw[:, b, :].rearrange("q (h j) -> q h j", h=2),
                sa_sb[b][:].rearrange("q h (j w) -> q h j w", w=PA)[:, :, :, P],
                sb_ps[:].rearrange("q (j h) -> q h j", h=2))
            nc.scalar.activation(stok_raw[:, b, :], stok_raw[:, b, :], Sqrt)
            nc.vector.reciprocal(stok_raw[:, b, :], stok_raw[:, b, :])
            # as split DVE/Pool (Pool is idle in this window)
            for c in range(TC):
                cc, hh = c // 2, c % 2
                eng = nc.gpsimd if c % 2 == 1 else nc.vector
                eng.tensor_scalar_mul(
                    as_sb[b][:, c, :], a_sb[b][:, c, :],
                    stok_raw[:, b, hh * (TC // 2) + cc:hh * (TC // 2) + cc + 1])

        def p2(b):
            alpha_ps = [ps.tile([128, 256], F32, tag="ps", name=f"al_ps{b}_{i}")
                        for i in range(DC)]
            for c in range(TC):
                for dc in range(DC):
                    nc.tensor.matmul(
                        alpha_ps[dc][:],
                        lhsT=b_sb[b][:, c, dc * 128:(dc + 1) * 128],
                        rhs=as_sb[b][:, c, :],
                        start=(c == 0), stop=(c == TC - 1))
            for dc in range(DC):
                nc.scalar.copy(alpha_sb[b][:, dc, :], alpha_ps[dc][:])
                nc.vector.tensor_mul(alsq_sb[b][:, dc, :],
                                     alpha_sb[b][:, dc, :],
                                     alpha_sb[b][:, dc, :])
            sal_ps = ps.tile([128, 2], F32, tag="ps", name=f"sal_ps{b}")
            for ec in range(2):
                for dc in range(DC):
                    nc.tensor.matmul(
                        sal_ps[:, ec:ec + 1],
                        lhsT=alsq_sb[b][:, dc, ec * 128:(ec + 1) * 128],
                        rhs=ones[:],
                        start=(dc == 0), stop=(dc == DC - 1))
            nc.vector.tensor_copy(sal_all[:, b, :], sal_ps[:])
            nc.scalar.activation(sal_all[:, b, :], sal_all[:, b, :], Sqrt)
            nc.vector.reciprocal(sal_all[:, b, :], sal_all[:, b, :])

        def p3(b):
            hm_sb = pscr.tile([128, 2, T], F16, tag="hm_sb", name=f"hm{b}")
            for t2 in range(2):
                for ec in range(2):
                    hp = ps.tile([128, 512], F32, tag="ps",
                                 name=f"hm_ps{b}_{ec}_{t2}")
                    for dc in range(DC):
                        nc.tensor.matmul(
                            hp[:],
                            lhsT=alpha_sb[b][:, dc, ec * 128:(ec + 1) * 128],
                            rhs=bT_sb[b][:, dc, t2 * 512:(t2 + 1) * 512],
                            start=(dc == 0), stop=(dc == DC - 1))
                    sl = slice(t2 * 512, t2 * 512 + 512)
                    if b == NB - 1 and ec == 1:
                        nc.vector.tensor_scalar_mul(
                            hm_sb[:, ec, sl], hp[:], sal_all[:, b, ec:ec + 1])
                    else:
                        nc.scalar.activation(hm_sb[:, ec, sl], hp[:], Copy,
                                             scale=sal_all[:, b, ec:ec + 1])
                    # prod/hmsq split DVE/Pool within each batch; the
                    # last batch runs all-DVE to shorten the tail
                    eng = nc.gpsimd if (t2 == 1 and b < NB - 1) else nc.vector
                    eng.tensor_mul(
                        prod_sb[b][:, ec, sl], aT_sb[b][:, ec, sl],
                        hm_sb[:, ec, sl])
                    eng.tensor_mul(
                        hmsq_sb[b][:, ec, sl], hm_sb[:, ec, sl],
                        hm_sb[:, ec, sl])

        def p4(b):
            fin_ps = [ps.tile([128, 2, (TC // 2) * P], F32, tag="ps",
                              name=f"fin_ps{b}_{i}") for i in range(2)]
            fin_ps_all[b] = fin_ps
            for c in range(TC):
                for q, srcq in enumerate((prod_sb[b], hmsq_sb[b])):
                    for ec in range(2):
                        nc.tensor.matmul(
                            fin_ps[c % 2][:, q, (c // 2) * P:(c // 2) * P + P],
                            lhsT=srcq[:, ec, c * 128:(c + 1) * 128],
                            rhs=w2t[:, ec, 0:P],
                            start=(ec == 0), stop=(ec == 1))
            for h in range(2):
                nc.vector.tensor_mul(
                    den_all[:, b, h, :].rearrange("q (j w) -> q j w", w=P),
                    sa_sb[b][:, h, :].rearrange(
                        "q (j w) -> q j w", w=PA)[:, :, 0:P],
                    fin_ps[h][:, 1, :].rearrange("q (j w) -> q j w", w=P))
            nc.scalar.activation(
                den_all[:, b].rearrange("q h jw -> q (h jw)"),
                den_all[:, b].rearrange("q h jw -> q (h jw)"), Sqrt)
            nc.vector.reciprocal(
                den_all[:, b].rearrange("q h jw -> q (h jw)"),
                den_all[:, b].rearrange("q h jw -> q (h jw)"))
            persp = pscr.tile([128, 2, (TC // 2) * P], F32, tag="persp",
                              name=f"persp{b}")
            for h in range(2):
                nc.vector.tensor_mul(
                    persp[:, h, :], fin_ps[h][:, 0, :], den_all[:, b, h, :])
                eng_dma = nc.sync if h == 0 else nc.scalar
                eng_dma.dma_start(
                    out=out_d.ap()[b].rearrange(
                        "(q j h) w -> q h j w", h=2, j=4)[:, h],
                    in_=persp[:, h, :].rearrange("q (j w) -> q j w", w=P))

        # ---- skewed emission: batch b runs phase s-b at step s ----
        # shallow-first within a step; p0's evacuations are emitted at
        # the END of the step so each engine's in-order stream reaches
        # older batches' ready work before transpose-gated evacs
        phases = [pload, p0, p1, p2, p3, p4]
        for s in range(len(phases) + NB - 1):
            for b in reversed(NBR):
                ph = s - b
                if 0 <= ph < len(phases):
                    phases[ph](b)

    nc.compile()
    return nc


_NC_CACHE = None


def _get_nc():
    global _NC_CACHE
    if _NC_CACHE is None:
        _NC_CACHE = build_kernel()
    return _NC_CACHE


def _make_in_maps(inp_a, inp_b, W):
    inp_a = np.ascontiguousarray(np.asarray(inp_a, dtype=np.float16))
    inp_b = np.ascontiguousarray(np.asarray(inp_b, dtype=np.float16))
    W = np.asarray(W, dtype=np.float32)
    w2t = np.ones((D, PA), dtype=np.float16)
    w2t[:, :P] = (W * W).T.astype(np.float16)
    return [
        {"a": inp_a[k * NB:(k + 1) * NB], "b": inp_b[k * NB:(k + 1) * NB],
         "w2t": w2t}
        for k in range(N_CORES)
    ]


def kernel(inp_a, inp_b, W):
    from concourse.bass_utils import run_bass_kernel_spmd
    nc = _get_nc()
    in_maps = _make_in_maps(inp_a, inp_b, W)
    res = run_bass_kernel_spmd(nc, in_maps, list(range(N_CORES)))
    persp = np.concatenate(
        [res.results[k]["out"] for k in range(N_CORES)], axis=0)
    return (persp, persp)


if __name__ == "__main__":
    rng = np.random.default_rng(0)
    inputs = {
        "inp_a": rng.standard_normal((B, T, D)).astype(np.float32),
        "inp_b": rng.standard_normal((B, T, D)).astype(np.float32),
        "W": rng.uniform(-0.05, 0.05, (P, 256)).astype(np.float32),
    }
    out = kernel(**inputs)
    print("ok", out[0].shape, out[0].dtype)
